# revision 1
# baseline (speedup 1.0000x reference)
"""Disentangled MHA (DeBERTa-style) Trainium2 Bass kernel.

Sharding: 16 heads across 8 cores (2 heads/core), batch kept local.
Per core: project q/k/v with a 128-column weight slice, build the
relative-position score bands, skew-gather them via a DRAM round trip,
softmax (transposed orientation, unnormalized-exp + fused Z column),
and PV matmul. Host concatenates the per-core 128-feature outputs.

B=4, S=512, DIM=1024, H=16, HD=64, MAX_REL=512.
"""

import numpy as np

import concourse.bass as bass
import concourse.bacc as bacc
import concourse.mybir as mybir
import concourse.tile as tile
from concourse.bass_utils import run_bass_kernel_spmd
from concourse.masks import make_identity

B, S, DIM, H, HD = 4, 512, 1024, 16, 64
T = B * S                      # 2048 tokens
R = 1024                       # 2 * att_span rel rows
HC = 2                         # heads per core
NCORES = 8
KC = DIM // 128                # contraction chunks
SCALE = float((HD * 3) ** (-0.5))
BAND = 640                     # skew band width (needs >= 512 + 127)

F32 = mybir.dt.float32
F32R = mybir.dt.float32r
F16 = mybir.dt.float16
AF = mybir.ActivationFunctionType
ALU = mybir.AluOpType


def _r32(ap):
    return ap.bitcast(F32R)


def build_nc():
    nc = bacc.Bacc("TRN2", target_bir_lowering=False, debug=False)

    xT_d = nc.dram_tensor("xT", [DIM, T], F16, kind="ExternalInput")
    relT_d = nc.dram_tensor("relT", [DIM, R], F16, kind="ExternalInput")
    W_d = {
        n: nc.dram_tensor(f"W{n}", [DIM, 128], F16, kind="ExternalInput")
        for n in "qkv"
    }
    b_d = {
        n: nc.dram_tensor(f"b{n}", [128, 1], F32, kind="ExternalInput")
        for n in "qkv"
    }
    out_d = nc.dram_tensor("out", [T, 128], F32, kind="ExternalOutput")

    with tile.TileContext(nc) as tc:
        _body(nc, tc, xT_d.ap(), relT_d.ap(),
              {n: W_d[n].ap() for n in "qkv"},
              {n: b_d[n].ap() for n in "qkv"},
              out_d.ap())
    nc.compile()
    return nc


def _body(nc, tc, xT, relT, W, bvec, out_d):
    from contextlib import ExitStack
    ctx = ExitStack()
    with ctx:
        singles = ctx.enter_context(tc.tile_pool(name="singles", bufs=1))

        # ---- Load inputs (spread across the three DMA queues) ----
        qeng = [nc.sync, nc.scalar, nc.gpsimd]
        xT_t = []
        for i in range(KC):
            t = singles.tile([128, T], F16, name=f"xT{i}")
            qeng[i % 3].dma_start(out=t, in_=xT[i * 128:(i + 1) * 128, :])
            xT_t.append(t)
        relT_t = []
        for i in range(KC):
            t = singles.tile([128, R], F16, name=f"relT{i}")
            qeng[(i + 1) % 3].dma_start(out=t, in_=relT[i * 128:(i + 1) * 128, :])
            relT_t.append(t)
        W_t = {}
        for wi, n in enumerate("qkv"):
            W_t[n] = []
            for i in range(KC):
                t = singles.tile([128, 128], F16, name=f"W{n}{i}")
                qeng[(i + wi) % 3].dma_start(
                    out=t, in_=W[n][i * 128:(i + 1) * 128, :])
                W_t[n].append(t)
        b_t = {}
        for n in "qkv":
            b_t[n] = singles.tile([128, 1], F32, name=f"b{n}")
            nc.gpsimd.dma_start(out=b_t[n], in_=bvec[n])

        ident = singles.tile([128, 128], F16, name="ident")
        make_identity(nc, ident)

        # ---- Phase A: projections (fp32r matmuls, fp16 outputs) ----
        q2T = singles.tile([128, T], F16, name="q2T")
        k2T = singles.tile([128, T], F16, name="k2T")
        v2T = singles.tile([128, T], F16, name="v2T")
        posk = singles.tile([128, R], F16, name="posk")
        posq = singles.tile([128, R], F16, name="posq")

        projs = [
            (q2T, xT_t, "q", T),
            (k2T, xT_t, "k", T),
            (posk, relT_t, "k", R),
            (posq, relT_t, "q", R),
            (v2T, xT_t, "v", T),
        ]
        with tc.tile_pool(name="psA", space="PSUM", bufs=2) as psA:
            for out_sb, rhs_tiles, wn, n_tot in projs:
                for nt in range(n_tot // 512):
                    ps = psA.tile([128, 512], F32, name="ps_proj", tag="ps_proj")
                    for kc in range(KC):
                        nc.tensor.matmul(
                            out=ps,
                            lhsT=W_t[wn][kc][:, :],
                            rhs=rhs_tiles[kc][:, nt * 512:(nt + 1) * 512],
                            start=(kc == 0), stop=(kc == KC - 1),
                        )
                    # cast f32->f16 + per-partition bias add
                    nc.scalar.activation(
                        out=out_sb[:, nt * 512:(nt + 1) * 512], in_=ps,
                        func=AF.Identity, bias=b_t[wn], scale=1.0,
                    )

            # ---- v_tok: transpose v2T to token-major, augmented ones col ----
            vtok = []
            for t in range(T // 128):
                vt = singles.tile([128, 130], F16, name=f"vtok{t}")
                vtok.append(vt)
            with tc.tile_pool(name="psVT", space="PSUM", bufs=2) as psVT:
                for t in range(T // 128):
                    pst = psVT.tile([128, 128], F16, name="ps_vt", tag="ps_vt")
                    nc.tensor.transpose(pst, v2T[:, t * 128:(t + 1) * 128], ident)
                    nc.vector.tensor_copy(vtok[t][:, 0:64], pst[:, 0:64])
                    nc.vector.tensor_copy(vtok[t][:, 65:129], pst[:, 64:128])
                    nc.gpsimd.memset(vtok[t][:, 64:65], 1.0)
                    nc.gpsimd.memset(vtok[t][:, 129:130], 1.0)

        # ---- Phase B ----
        band_dram = ctx.enter_context(
            tc.tile_pool(name="bands", space="DRAM", bufs=1))
        sb_band = ctx.enter_context(tc.tile_pool(name="sb_band", bufs=3))
        sb_work = ctx.enter_context(tc.tile_pool(name="sb_work", bufs=4))
        sb_out = ctx.enter_context(tc.tile_pool(name="sb_out", bufs=3))
        ps_band_pool = ctx.enter_context(
            tc.tile_pool(name="psBand", space="PSUM", bufs=2))
        ps_qk_pool = ctx.enter_context(
            tc.tile_pool(name="psQK", space="PSUM", bufs=2))
        ps_pv_pool = ctx.enter_context(
            tc.tile_pool(name="psPV", space="PSUM", bufs=2))

        copy_flip = [0]

        def psum_to_sbuf_f16(dst, src):
            # alternate engines to balance DVE/ACT load
            if copy_flip[0] % 2 == 0:
                nc.vector.tensor_copy(dst, src)
            else:
                nc.scalar.copy(dst, src)
            copy_flip[0] += 1

        # --- B1: score bands for one batch: matmul -> sbuf -> dram ---
        c2p_bd = {}         # (b, h) -> dram tile [512, 1024] (pitch-1024 skew)
        p2c_bd = {}         # (b, h) -> dram tile [128, 4*BAND]

        def emit_b1(b):
            csb = {h: sb_band.tile([128, 4 * BAND], F16, name=f"c2p_sb{h}",
                                   tag=f"c2p_sb{h}", bufs=2) for h in range(HC)}
            psb = {h: sb_band.tile([128, 4 * BAND], F16, name=f"p2c_sb{h}",
                                   tag=f"p2c_sb{h}", bufs=2) for h in range(HC)}
            for blk in range(4):
                c0 = 128 * (3 - blk)
                cs = slice(b * 512 + blk * 128, b * 512 + (blk + 1) * 128)
                for src2T, pos, stage in ((q2T, posk, csb), (k2T, posq, psb)):
                    pss = []
                    for h in range(HC):   # adjacent h matmuls -> PE row packing
                        hs = slice(h * 64, (h + 1) * 64)
                        ps = ps_band_pool.tile([128, BAND], F32,
                                               name="ps_band", tag="ps_band")
                        nc.tensor.matmul(
                            out=ps[:, 0:512], lhsT=src2T[hs, cs],
                            rhs=pos[hs, c0:c0 + 512], start=True, stop=True)
                        nc.tensor.matmul(
                            out=ps[:, 512:BAND], lhsT=src2T[hs, cs],
                            rhs=pos[hs, c0 + 512:c0 + BAND],
                            start=True, stop=True)
                        pss.append(ps)
                    for h in range(HC):
                        psum_to_sbuf_f16(
                            stage[h][:, blk * BAND:(blk + 1) * BAND], pss[h])
            for h in range(HC):
                # c2p: strided write into a [512, 1024]-pitch buffer so the
                # transposed skew read is a single 2D AP (offset q*1023+k+512)
                bdr = band_dram.tile([512, 1024], F16, name=f"c2pb_{b}{h}",
                                     tag=f"c2p_dram_{b}{h}", bufs=1)
                dst = bass.AP(bdr.tensor, bdr.offset + 384,
                              [[1024, 128], [130944, 4], [1, BAND]])
                nc.sync.dma_start(
                    out=dst, in_=csb[h].rearrange("p (g j) -> p g j", g=4))
                c2p_bd[(b, h)] = bdr
                # p2c: flat [128, 4*BAND]; skew read offset ki*2559+kb*640+128+q
                bdr = band_dram.tile([128, 4 * BAND], F16, name=f"p2cb_{b}{h}",
                                     tag=f"p2c_dram_{b}{h}", bufs=1)
                nc.scalar.dma_start(out=bdr, in_=psb[h])
                p2c_bd[(b, h)] = bdr

        # --- B2: attention for one batch ---
        # constant exp bias keeps f16 E and the f16-transposed Z in range;
        # it cancels exactly in the final E@v / Z normalization
        exp_bias = singles.tile([128, 1], F32, name="exp_bias")
        nc.gpsimd.memset(exp_bias, -4.0)

        def emit_b2(b):
            ostage = sb_out.tile([128, 512], F32, name="ostage", tag="ostage")
            for h in range(HC):
                hs = slice(h * 64, (h + 1) * 64)
                ps_pv = ps_pv_pool.tile([65, 512], F32, name="ps_pv", tag="ps_pv")
                for kb in range(4):
                    ks = slice(b * 512 + kb * 128, b * 512 + (kb + 1) * 128)
                    # qkT: [k 128, q 512]
                    ps_qk = ps_qk_pool.tile([128, 512], F32,
                                            name="ps_qk", tag="ps_qk")
                    nc.tensor.matmul(
                        out=ps_qk, lhsT=k2T[hs, ks],
                        rhs=q2T[hs, b * 512:(b + 1) * 512],
                        start=True, stop=True)

                    # c2pT: one transposed skew read over the full-pitch band
                    t_sb = sb_work.tile([128, 512], F16, name="t_sb",
                                        tag="t_sb", bufs=16)
                    bdr = c2p_bd[(b, h)]
                    src = bass.AP(bdr.tensor, bdr.offset + 512 + 128 * kb,
                                  [[1023, 512], [1, 128]])
                    nc.sync.dma_start_transpose(out=t_sb, in_=src)
                    # p2cT: accumulate plain skew read
                    bdr = p2c_bd[(b, h)]
                    src = bass.AP(bdr.tensor, bdr.offset + kb * BAND + 128,
                                  [[4 * BAND - 1, 128], [1, 512]])
                    nc.gpsimd.dma_start(out=t_sb, in_=src, accum_op=ALU.add)

                    # scores + exp (unnormalized, transposed)
                    s_sb = sb_work.tile([128, 512], F16, name="s_sb",
                                        tag="s_sb", bufs=12)
                    nc.vector.tensor_tensor(out=s_sb, in0=t_sb, in1=ps_qk,
                                            op=ALU.add)
                    eT = sb_work.tile([128, 512], F16, name="eT", tag="eT",
                                      bufs=12)
                    nc.scalar.activation(out=eT, in_=s_sb, func=AF.Exp,
                                         scale=SCALE, bias=exp_bias)
                    # PV with stationary [v|1]: psum [65, 512] = (v|1)^T @ E^T
                    nc.tensor.matmul(
                        out=ps_pv, lhsT=vtok[b * 4 + kb][:, h * 65:h * 65 + 65],
                        rhs=eT, start=(kb == 0), stop=(kb == 3))

                # --- finalize: out^T [65, 512] -> transpose -> /Z -> stage ---
                o2T = sb_work.tile([65, 512], F16, name="o2T", tag="o2T")
                nc.scalar.copy(o2T, ps_pv)
                for qc in range(4):
                    psT = ps_band_pool.tile([128, 65], F16, name="psT",
                                            tag="ps_band")
                    nc.tensor.transpose(psT, o2T[:, qc * 128:(qc + 1) * 128],
                                        ident[0:65, 0:65])
                    zrec = sb_work.tile([128, 1], F32, name="zrec",
                                        tag="zrec", bufs=8)
                    nc.vector.reciprocal(zrec, psT[:, 64:65])
                    nc.vector.tensor_scalar_mul(
                        ostage[:, qc * 128 + h * 64:qc * 128 + (h + 1) * 64],
                        psT[:, 0:64], zrec)
            # one merged output write per batch
            dst = bass.AP(out_d.tensor, out_d.offset + b * 65536,
                          [[128, 128], [16384, 4], [1, 128]])
            nc.scalar.dma_start(
                out=dst, in_=ostage.rearrange("p (g j) -> p g j", g=4))

        # software-pipelined emission: bands stay two batches ahead of the
        # attention consuming them, so no queue's FIFO head blocks on work
        # that hasn't been produced yet
        emit_b1(0)
        emit_b1(1)
        emit_b2(0)
        emit_b1(2)
        emit_b2(1)
        emit_b1(3)
        emit_b2(2)
        emit_b2(3)


_NC_CACHE = None


def _get_nc():
    global _NC_CACHE
    if _NC_CACHE is None:
        _NC_CACHE = build_nc()
    return _NC_CACHE


def make_in_maps(inputs):
    x = np.asarray(inputs["x"], np.float32)
    rel = np.asarray(inputs["rel_embeddings"], np.float32)
    Wq = np.asarray(inputs["Wq"], np.float32)
    Wk = np.asarray(inputs["Wk"], np.float32)
    Wv = np.asarray(inputs["Wv"], np.float32)
    bq = np.asarray(inputs["bq"], np.float32)
    bk = np.asarray(inputs["bk"], np.float32)
    bv = np.asarray(inputs["bv"], np.float32)

    xT = np.ascontiguousarray(x.reshape(T, DIM).T).astype(np.float16)
    relT = np.ascontiguousarray(rel[::-1].T).astype(np.float16)
    in_maps = []
    for c in range(NCORES):
        sl = slice(c * 128, (c + 1) * 128)
        in_maps.append({
            "xT": xT,
            "relT": relT,
            "Wq": np.ascontiguousarray(Wq[:, sl]).astype(np.float16),
            "Wk": np.ascontiguousarray(Wk[:, sl]).astype(np.float16),
            "Wv": np.ascontiguousarray(Wv[:, sl]).astype(np.float16),
            "bq": np.ascontiguousarray(bq[sl]).reshape(128, 1),
            "bk": np.ascontiguousarray(bk[sl]).reshape(128, 1),
            "bv": np.ascontiguousarray(bv[sl]).reshape(128, 1),
        })
    return in_maps


def kernel(**inputs):
    nc = _get_nc()
    in_maps = make_in_maps(inputs)
    res = run_bass_kernel_spmd(nc, in_maps, list(range(NCORES))).results
    out = np.concatenate([res[c]["out"] for c in range(NCORES)], axis=1)
    return out.reshape(B, S, DIM).astype(np.float32)



# revision 3
# speedup vs baseline: 1.4555x; 1.4555x over previous
"""Disentangled MHA (DeBERTa-style) Trainium2 Bass kernel, v2.

Sharding: 16 heads across 8 cores (2 heads/core), batch kept local.
Per core: project q/k/v with a 128-column weight slice, build banded
c2p/p2c score bands, round-trip them through DRAM as flat [128,2560]
tiles (one write + one 3D skew-read each), then fold the c2p transpose
and the p2c add directly into the QK PSUM with identity matmuls:
PSUM = qk + c2p^T + p2c, exp straight off PSUM, PV matmul with an
augmented [v|1] stationary for the fused softmax denominator.

B=4, S=512, DIM=1024, H=16, HD=64, MAX_REL=512.
"""

import numpy as np

import concourse.bass as bass
import concourse.bacc as bacc
import concourse.mybir as mybir
import concourse.tile as tile
from concourse.bass_utils import run_bass_kernel_spmd
from concourse.masks import make_identity

B, S, DIM, H, HD = 4, 512, 1024, 16, 64
T = B * S                      # 2048 tokens
R = 1024                       # 2 * att_span rel rows
HC = 2                         # heads per core
NCORES = 8
KC = DIM // 128                # contraction chunks
SCALE = float((HD * 3) ** (-0.5))
BAND = 640                     # skew band width (needs >= 512 + 127)

F32 = mybir.dt.float32
F16 = mybir.dt.float16
AF = mybir.ActivationFunctionType
ALU = mybir.AluOpType


def build_nc():
    nc = bacc.Bacc("TRN2", target_bir_lowering=False, debug=False)

    xT_d = nc.dram_tensor("xT", [DIM, T], F16, kind="ExternalInput")
    relT_d = nc.dram_tensor("relT", [DIM, R], F16, kind="ExternalInput")
    W_d = {
        n: nc.dram_tensor(f"W{n}", [DIM, 128], F16, kind="ExternalInput")
        for n in "qkv"
    }
    b_d = {
        n: nc.dram_tensor(f"b{n}", [128, 1], F32, kind="ExternalInput")
        for n in "qkv"
    }
    out_d = nc.dram_tensor("out", [T, 128], F32, kind="ExternalOutput")

    with tile.TileContext(nc) as tc:
        _body(nc, tc, xT_d.ap(), relT_d.ap(),
              {n: W_d[n].ap() for n in "qkv"},
              {n: b_d[n].ap() for n in "qkv"},
              out_d.ap())
    nc.compile()
    return nc


def _body(nc, tc, xT, relT, W, bvec, out_d):
    from contextlib import ExitStack
    ctx = ExitStack()
    with ctx:
        singles = ctx.enter_context(tc.tile_pool(name="singles", bufs=1))

        # ---- Load inputs (spread across the three DMA queues) ----
        qeng = [nc.sync, nc.scalar, nc.gpsimd]
        xT_t = []
        for i in range(KC):
            t = singles.tile([128, T], F16, name=f"xT{i}")
            qeng[i % 3].dma_start(out=t, in_=xT[i * 128:(i + 1) * 128, :])
            xT_t.append(t)
        relT_t = []
        for i in range(KC):
            t = singles.tile([128, R], F16, name=f"relT{i}")
            qeng[(i + 1) % 3].dma_start(out=t, in_=relT[i * 128:(i + 1) * 128, :])
            relT_t.append(t)
        W_t = {}
        for wi, n in enumerate("qkv"):
            W_t[n] = []
            for i in range(KC):
                t = singles.tile([128, 128], F16, name=f"W{n}{i}")
                qeng[(i + wi) % 3].dma_start(
                    out=t, in_=W[n][i * 128:(i + 1) * 128, :])
                W_t[n].append(t)
        b_t = {}
        for n in "qkv":
            b_t[n] = singles.tile([128, 1], F32, name=f"b{n}")
            nc.gpsimd.dma_start(out=b_t[n], in_=bvec[n])

        ident = singles.tile([128, 128], F16, name="ident")
        make_identity(nc, ident)

        # ---- Phase A: projections (f16 matmuls, fp32 psum, f16 out) ----
        q2T = singles.tile([128, T], F16, name="q2T")
        k2T = singles.tile([128, T], F16, name="k2T")
        v2T = singles.tile([128, T], F16, name="v2T")
        posk = singles.tile([128, R], F16, name="posk")
        posq = singles.tile([128, R], F16, name="posq")

        projs = [
            (q2T, xT_t, "q", T),
            (k2T, xT_t, "k", T),
            (posk, relT_t, "k", R),
            (posq, relT_t, "q", R),
            (v2T, xT_t, "v", T),
        ]
        with tc.tile_pool(name="psA", space="PSUM", bufs=2) as psA:
            for out_sb, rhs_tiles, wn, n_tot in projs:
                for nt in range(n_tot // 512):
                    ps = psA.tile([128, 512], F32, name="ps_proj", tag="ps_proj")
                    for kc in range(KC):
                        nc.tensor.matmul(
                            out=ps,
                            lhsT=W_t[wn][kc][:, :],
                            rhs=rhs_tiles[kc][:, nt * 512:(nt + 1) * 512],
                            start=(kc == 0), stop=(kc == KC - 1),
                        )
                    # cast f32->f16 + per-partition bias add
                    nc.scalar.activation(
                        out=out_sb[:, nt * 512:(nt + 1) * 512], in_=ps,
                        func=AF.Identity, bias=b_t[wn], scale=1.0,
                    )

            # ---- v_tok: transpose v2T to token-major, augmented ones col ----
            vtok = []
            for t in range(T // 128):
                vt = singles.tile([128, 130], F16, name=f"vtok{t}")
                vtok.append(vt)
            with tc.tile_pool(name="psVT", space="PSUM", bufs=2) as psVT:
                for t in range(T // 128):
                    pst = psVT.tile([128, 128], F16, name="ps_vt", tag="ps_vt")
                    nc.tensor.transpose(pst, v2T[:, t * 128:(t + 1) * 128], ident)
                    nc.vector.tensor_copy(vtok[t][:, 0:64], pst[:, 0:64])
                    nc.vector.tensor_copy(vtok[t][:, 65:129], pst[:, 64:128])
                    nc.gpsimd.memset(vtok[t][:, 64:65], 1.0)
                    nc.gpsimd.memset(vtok[t][:, 129:130], 1.0)

        # ---- Phase B pools ----
        band_dram = ctx.enter_context(
            tc.tile_pool(name="bands", space="DRAM", bufs=1))
        sb_band = ctx.enter_context(tc.tile_pool(name="sb_band", bufs=2))
        sb_read = ctx.enter_context(tc.tile_pool(name="sb_read", bufs=3))
        sb_work = ctx.enter_context(tc.tile_pool(name="sb_work", bufs=4))
        sb_out = ctx.enter_context(tc.tile_pool(name="sb_out", bufs=3))
        ps_band_pool = ctx.enter_context(
            tc.tile_pool(name="psBand", space="PSUM", bufs=2))
        ps_qk_pool = ctx.enter_context(
            tc.tile_pool(name="psQK", space="PSUM", bufs=2))
        ps_pv_pool = ctx.enter_context(
            tc.tile_pool(name="psPV", space="PSUM", bufs=2))

        copy_flip = [0]

        def psum_to_sbuf_f16(dst, src):
            # alternate engines to balance DVE/ACT load
            if copy_flip[0] % 2 == 0:
                nc.vector.tensor_copy(dst, src)
            else:
                nc.scalar.copy(dst, src)
            copy_flip[0] += 1

        # constant exp bias keeps the f16 unnormalized-exp in range; it
        # cancels exactly in the final E@v / Z normalization
        exp_bias = singles.tile([128, 1], F32, name="exp_bias")
        nc.gpsimd.memset(exp_bias, -4.0)

        units = [(b, h) for b in range(B) for h in range(HC)]
        c2p_bd = {}
        p2c_bd = {}

        # --- B1 for unit (b,h): band matmuls -> sbuf -> one flat write each
        def emit_b1(u):
            b, h = units[u]
            hs = slice(h * 64, (h + 1) * 64)
            csb = sb_band.tile([128, 4 * BAND], F16, name=f"csb{u}", tag="csb")
            psb = sb_band.tile([128, 4 * BAND], F16, name=f"psb{u}", tag="psb")
            for g in range(4):
                c0 = 128 * (3 - g)
                cs = slice(b * 512 + g * 128, b * 512 + (g + 1) * 128)
                for src2T, pos, stage in ((q2T, posk, csb), (k2T, posq, psb)):
                    ps = ps_band_pool.tile([128, BAND], F32,
                                           name="ps_band", tag="ps_band")
                    nc.tensor.matmul(
                        out=ps[:, 0:512], lhsT=src2T[hs, cs],
                        rhs=pos[hs, c0:c0 + 512], start=True, stop=True)
                    nc.tensor.matmul(
                        out=ps[:, 512:BAND], lhsT=src2T[hs, cs],
                        rhs=pos[hs, c0 + 512:c0 + BAND],
                        start=True, stop=True)
                    psum_to_sbuf_f16(stage[:, g * BAND:(g + 1) * BAND], ps)
            bdr = band_dram.tile([128, 4 * BAND], F16, name=f"c2pb_{u}",
                                 tag=f"c2p_dram_{u}", bufs=1)
            nc.sync.dma_start(out=bdr, in_=csb)
            c2p_bd[u] = bdr
            bdr = band_dram.tile([128, 4 * BAND], F16, name=f"p2cb_{u}",
                                 tag=f"p2c_dram_{u}", bufs=1)
            nc.scalar.dma_start(out=bdr, in_=psb)
            p2c_bd[u] = bdr

        # --- B2 for unit (b,h): skew reads + attention
        ostage = {}

        def emit_b2(u):
            b, h = units[u]
            hs = slice(h * 64, (h + 1) * 64)
            # one 3D skew read per band: [p, g, j=128..639] with j=base+idx
            ct = sb_read.tile([128, 2048], F16, name="ct", tag="ct")
            bdr = c2p_bd[u]
            src = bass.AP(bdr.tensor, bdr.offset + 128,
                          [[4 * BAND - 1, 128], [BAND, 4], [1, 512]])
            nc.sync.dma_start(out=ct.rearrange("p (g j) -> p g j", g=4),
                              in_=src)
            pt = sb_read.tile([128, 2048], F16, name="pt", tag="pt")
            bdr = p2c_bd[u]
            src = bass.AP(bdr.tensor, bdr.offset + 128,
                          [[4 * BAND - 1, 128], [BAND, 4], [1, 512]])
            nc.gpsimd.dma_start(out=pt.rearrange("p (g j) -> p g j", g=4),
                                in_=src)

            if h == 0:
                ostage[b] = sb_out.tile([128, 512], F32, name=f"ostage{b}",
                                        tag="ostage")
            ps_pv = ps_pv_pool.tile([65, 512], F32, name="ps_pv", tag="ps_pv")
            eTs = []
            for kb in range(4):
                ks = slice(b * 512 + kb * 128, b * 512 + (kb + 1) * 128)
                # PSUM = qk + c2p^T (4 transposed chunks) + p2c (copy-accum)
                ps_qk = ps_qk_pool.tile([128, 512], F32,
                                        name="ps_qk", tag="ps_qk")
                nc.tensor.matmul(
                    out=ps_qk, lhsT=k2T[hs, ks],
                    rhs=q2T[hs, b * 512:(b + 1) * 512],
                    start=True, stop=False)
                for g in range(4):
                    nc.tensor.matmul(
                        out=ps_qk[:, g * 128:(g + 1) * 128],
                        lhsT=ct[:, g * 512 + kb * 128:g * 512 + kb * 128 + 128],
                        rhs=ident,
                        start=False, stop=False)
                nc.tensor.matmul(
                    out=ps_qk, lhsT=ident,
                    rhs=pt[:, kb * 512:(kb + 1) * 512],
                    start=False, stop=True)
                # unnormalized exp straight off PSUM
                eT = sb_work.tile([128, 512], F16, name="eT", tag="eT",
                                  bufs=6)
                nc.scalar.activation(out=eT, in_=ps_qk, func=AF.Exp,
                                     scale=SCALE, bias=exp_bias)
                eTs.append(eT)
            for kb in range(4):
                nc.tensor.matmul(
                    out=ps_pv, lhsT=vtok[b * 4 + kb][:, h * 65:h * 65 + 65],
                    rhs=eTs[kb], start=(kb == 0), stop=(kb == 3))

            # --- finalize: out^T [65, 512] -> transpose -> /Z -> stage ---
            o2T = sb_work.tile([65, 512], F16, name="o2T", tag="o2T")
            nc.scalar.copy(o2T, ps_pv)
            for qc in range(4):
                psT = ps_band_pool.tile([128, 65], F16, name="psT",
                                        tag="ps_band")
                nc.tensor.transpose(psT, o2T[:, qc * 128:(qc + 1) * 128],
                                    ident[0:65, 0:65])
                zrec = sb_work.tile([128, 1], F32, name="zrec",
                                    tag="zrec", bufs=8)
                nc.vector.reciprocal(zrec, psT[:, 64:65])
                nc.vector.tensor_scalar_mul(
                    ostage[b][:, qc * 128 + h * 64:qc * 128 + (h + 1) * 64],
                    psT[:, 0:64], zrec)
            if h == HC - 1:
                # one merged output write per batch
                dst = bass.AP(out_d.tensor, out_d.offset + b * 65536,
                              [[128, 128], [16384, 4], [1, 128]])
                nc.scalar.dma_start(
                    out=dst, in_=ostage[b].rearrange("p (g j) -> p g j", g=4))

        # software pipeline: band production stays two units ahead of the
        # attention consuming it, so reads never wait on unissued writes
        emit_b1(0)
        emit_b1(1)
        emit_b1(2)
        emit_b2(0)
        emit_b1(3)
        emit_b2(1)
        emit_b1(4)
        emit_b2(2)
        emit_b1(5)
        emit_b2(3)
        emit_b1(6)
        emit_b2(4)
        emit_b1(7)
        emit_b2(5)
        emit_b2(6)
        emit_b2(7)


_NC_CACHE = None


def _get_nc():
    global _NC_CACHE
    if _NC_CACHE is None:
        _NC_CACHE = build_nc()
    return _NC_CACHE


def make_in_maps(inputs):
    x = np.asarray(inputs["x"], np.float32)
    rel = np.asarray(inputs["rel_embeddings"], np.float32)
    Wq = np.asarray(inputs["Wq"], np.float32)
    Wk = np.asarray(inputs["Wk"], np.float32)
    Wv = np.asarray(inputs["Wv"], np.float32)
    bq = np.asarray(inputs["bq"], np.float32)
    bk = np.asarray(inputs["bk"], np.float32)
    bv = np.asarray(inputs["bv"], np.float32)

    xT = np.ascontiguousarray(x.reshape(T, DIM).T).astype(np.float16)
    relT = np.ascontiguousarray(rel[::-1].T).astype(np.float16)
    in_maps = []
    for c in range(NCORES):
        sl = slice(c * 128, (c + 1) * 128)
        in_maps.append({
            "xT": xT,
            "relT": relT,
            "Wq": np.ascontiguousarray(Wq[:, sl]).astype(np.float16),
            "Wk": np.ascontiguousarray(Wk[:, sl]).astype(np.float16),
            "Wv": np.ascontiguousarray(Wv[:, sl]).astype(np.float16),
            "bq": np.ascontiguousarray(bq[sl]).reshape(128, 1),
            "bk": np.ascontiguousarray(bk[sl]).reshape(128, 1),
            "bv": np.ascontiguousarray(bv[sl]).reshape(128, 1),
        })
    return in_maps


def kernel(**inputs):
    nc = _get_nc()
    in_maps = make_in_maps(inputs)
    res = run_bass_kernel_spmd(nc, in_maps, list(range(NCORES))).results
    out = np.concatenate([res[c]["out"] for c in range(NCORES)], axis=1)
    return out.reshape(B, S, DIM).astype(np.float32)


# revision 5
# speedup vs baseline: 1.5665x; 1.0762x over previous
"""Disentangled MHA (DeBERTa-style) Trainium2 Bass kernel, v2.

Sharding: 16 heads across 8 cores (2 heads/core), batch kept local.
Per core: project q/k/v with a 128-column weight slice, build banded
c2p/p2c score bands, round-trip them through DRAM as flat [128,2560]
tiles (one write + one 3D skew-read each), then fold the c2p transpose
and the p2c add directly into the QK PSUM with identity matmuls:
PSUM = qk + c2p^T + p2c, exp straight off PSUM, PV matmul with an
augmented [v|1] stationary for the fused softmax denominator.

B=4, S=512, DIM=1024, H=16, HD=64, MAX_REL=512.
"""

import numpy as np

import concourse.bass as bass
import concourse.bacc as bacc
import concourse.mybir as mybir
import concourse.tile as tile
from concourse.bass_utils import run_bass_kernel_spmd
from concourse.masks import make_identity

B, S, DIM, H, HD = 4, 512, 1024, 16, 64
T = B * S                      # 2048 tokens
R = 1024                       # 2 * att_span rel rows
HC = 2                         # heads per core
NCORES = 8
KC = DIM // 128                # contraction chunks
SCALE = float((HD * 3) ** (-0.5))
BAND = 640                     # skew band width (needs >= 512 + 127)

F32 = mybir.dt.float32
F16 = mybir.dt.float16
AF = mybir.ActivationFunctionType
ALU = mybir.AluOpType


def build_nc():
    nc = bacc.Bacc("TRN2", target_bir_lowering=False, debug=False)

    xT_d = nc.dram_tensor("xT", [DIM, T], F16, kind="ExternalInput")
    relT_d = nc.dram_tensor("relT", [DIM, R], F16, kind="ExternalInput")
    W_d = {
        n: nc.dram_tensor(f"W{n}", [DIM, 128], F16, kind="ExternalInput")
        for n in "qkv"
    }
    b_d = {
        n: nc.dram_tensor(f"b{n}", [128, 1], F32, kind="ExternalInput")
        for n in "qkv"
    }
    out_d = nc.dram_tensor("out", [T, 128], F32, kind="ExternalOutput")

    with tile.TileContext(nc) as tc:
        _body(nc, tc, xT_d.ap(), relT_d.ap(),
              {n: W_d[n].ap() for n in "qkv"},
              {n: b_d[n].ap() for n in "qkv"},
              out_d.ap())
    nc.compile()
    return nc


def _body(nc, tc, xT, relT, W, bvec, out_d):
    from contextlib import ExitStack
    ctx = ExitStack()
    with ctx:
        singles = ctx.enter_context(tc.tile_pool(name="singles", bufs=1))

        # ---- Load inputs (spread across the three DMA queues) ----
        qeng = [nc.sync, nc.scalar, nc.gpsimd]
        xT_t = []
        for i in range(KC):
            t = singles.tile([128, T], F16, name=f"xT{i}")
            qeng[i % 3].dma_start(out=t, in_=xT[i * 128:(i + 1) * 128, :])
            xT_t.append(t)
        relT_t = []
        for i in range(KC):
            t = singles.tile([128, R], F16, name=f"relT{i}")
            qeng[(i + 1) % 3].dma_start(out=t, in_=relT[i * 128:(i + 1) * 128, :])
            relT_t.append(t)
        W_t = {}
        for wi, n in enumerate("qkv"):
            W_t[n] = []
            for i in range(KC):
                t = singles.tile([128, 128], F16, name=f"W{n}{i}")
                qeng[(i + wi) % 3].dma_start(
                    out=t, in_=W[n][i * 128:(i + 1) * 128, :])
                W_t[n].append(t)
        b_t = {}
        for n in "qkv":
            b_t[n] = singles.tile([128, 1], F32, name=f"b{n}")
            nc.gpsimd.dma_start(out=b_t[n], in_=bvec[n])

        ident = singles.tile([128, 128], F16, name="ident")
        make_identity(nc, ident)

        # ---- Phase A: projections (f16 matmuls, fp32 psum, f16 out) ----
        q2T = singles.tile([128, T], F16, name="q2T")
        k2T = singles.tile([128, T], F16, name="k2T")
        v2T = singles.tile([128, T], F16, name="v2T")
        posk = singles.tile([128, R], F16, name="posk")
        posq = singles.tile([128, R], F16, name="posq")

        projs = [
            (q2T, xT_t, "q", T),
            (k2T, xT_t, "k", T),
            (posk, relT_t, "k", R),
            (posq, relT_t, "q", R),
            (v2T, xT_t, "v", T),
        ]
        cast_flip = [0]
        with tc.tile_pool(name="psA", space="PSUM", bufs=2) as psA:
            for out_sb, rhs_tiles, wn, n_tot in projs:
                for nt in range(n_tot // 512):
                    ps = psA.tile([128, 512], F32, name="ps_proj", tag="ps_proj")
                    for kc in range(KC):
                        nc.tensor.matmul(
                            out=ps,
                            lhsT=W_t[wn][kc][:, :],
                            rhs=rhs_tiles[kc][:, nt * 512:(nt + 1) * 512],
                            start=(kc == 0), stop=(kc == KC - 1),
                        )
                    # cast f32->f16 + per-partition bias add (ACT/DVE split)
                    dst = out_sb[:, nt * 512:(nt + 1) * 512]
                    if cast_flip[0] % 2 == 0:
                        nc.scalar.activation(out=dst, in_=ps,
                                             func=AF.Identity, bias=b_t[wn],
                                             scale=1.0)
                    else:
                        nc.vector.tensor_scalar_add(dst, ps, b_t[wn])
                    cast_flip[0] += 1

            # ---- v_tok: transpose v2T to token-major, augmented ones col ----
            vtok = []
            for t in range(T // 128):
                vt = singles.tile([128, 130], F16, name=f"vtok{t}")
                vtok.append(vt)
            with tc.tile_pool(name="psVT", space="PSUM", bufs=2) as psVT:
                for t in range(T // 128):
                    pst = psVT.tile([128, 128], F16, name="ps_vt", tag="ps_vt")
                    nc.tensor.transpose(pst, v2T[:, t * 128:(t + 1) * 128], ident)
                    nc.vector.tensor_copy(vtok[t][:, 0:64], pst[:, 0:64])
                    nc.vector.tensor_copy(vtok[t][:, 65:129], pst[:, 64:128])
                    nc.gpsimd.memset(vtok[t][:, 64:65], 1.0)
                    nc.gpsimd.memset(vtok[t][:, 129:130], 1.0)

        # ---- Phase B pools ----
        band_dram = ctx.enter_context(
            tc.tile_pool(name="bands", space="DRAM", bufs=1))
        sb_band = ctx.enter_context(tc.tile_pool(name="sb_band", bufs=2))
        sb_read = ctx.enter_context(tc.tile_pool(name="sb_read", bufs=3))
        sb_work = ctx.enter_context(tc.tile_pool(name="sb_work", bufs=4))
        sb_out = ctx.enter_context(tc.tile_pool(name="sb_out", bufs=3))

        copy_flip = [0]

        def psum_to_sbuf_f16(dst, src):
            # alternate engines to balance DVE/ACT load
            if copy_flip[0] % 2 == 0:
                nc.vector.tensor_copy(dst, src)
            else:
                nc.scalar.copy(dst, src)
            copy_flip[0] += 1

        # constant exp bias keeps the f16 unnormalized-exp in range; it
        # cancels exactly in the final E@v / Z normalization
        exp_bias = singles.tile([128, 1], F32, name="exp_bias")
        nc.gpsimd.memset(exp_bias, -4.0)

        units = [(b, h) for b in range(B) for h in range(HC)]
        c2p_bd = {}
        p2c_bd = {}

        # --- B1: all band matmuls dense (keeps PE warm), writes stream out
        with tc.tile_pool(name="psBand", space="PSUM", bufs=2) as ps_band_pool:
            for u in range(len(units)):
                b, h = units[u]
                hs = slice(h * 64, (h + 1) * 64)
                csb = sb_band.tile([128, 4 * BAND], F16, name=f"csb{u}",
                                   tag="csb")
                psb = sb_band.tile([128, 4 * BAND], F16, name=f"psb{u}",
                                   tag="psb")
                for g in range(4):
                    c0 = 128 * (3 - g)
                    cs = slice(b * 512 + g * 128, b * 512 + (g + 1) * 128)
                    for src2T, pos, stage in ((q2T, posk, csb),
                                              (k2T, posq, psb)):
                        ps = ps_band_pool.tile([128, BAND], F32,
                                               name="ps_band", tag="ps_band")
                        nc.tensor.matmul(
                            out=ps[:, 0:512], lhsT=src2T[hs, cs],
                            rhs=pos[hs, c0:c0 + 512], start=True, stop=True)
                        nc.tensor.matmul(
                            out=ps[:, 512:BAND], lhsT=src2T[hs, cs],
                            rhs=pos[hs, c0 + 512:c0 + BAND],
                            start=True, stop=True)
                        psum_to_sbuf_f16(stage[:, g * BAND:(g + 1) * BAND], ps)
                bdr = band_dram.tile([128, 4 * BAND], F16, name=f"c2pb_{u}",
                                     tag=f"c2p_dram_{u}", bufs=1)
                nc.sync.dma_start(out=bdr, in_=csb)
                c2p_bd[u] = bdr
                bdr = band_dram.tile([128, 4 * BAND], F16, name=f"p2cb_{u}",
                                     tag=f"p2c_dram_{u}", bufs=1)
                nc.scalar.dma_start(out=bdr, in_=psb)
                p2c_bd[u] = bdr

        # --- B2: skew reads + attention, [128,1024] two-bank psum halves
        ps_qk_pool = ctx.enter_context(
            tc.tile_pool(name="psQK", space="PSUM", bufs=2))
        ps_pv_pool = ctx.enter_context(
            tc.tile_pool(name="psPV", space="PSUM", bufs=2))
        ps_t_pool = ctx.enter_context(
            tc.tile_pool(name="psT", space="PSUM", bufs=2))
        ostage = {}

        for u in range(len(units)):
            b, h = units[u]
            hs = slice(h * 64, (h + 1) * 64)
            # one 3D skew read per band: [p, g, j=128..639] with j=base+idx
            ct = sb_read.tile([128, 2048], F16, name="ct", tag="ct")
            bdr = c2p_bd[u]
            src = bass.AP(bdr.tensor, bdr.offset + 128,
                          [[4 * BAND - 1, 128], [BAND, 4], [1, 512]])
            nc.sync.dma_start(out=ct.rearrange("p (g j) -> p g j", g=4),
                              in_=src)
            pt = sb_read.tile([128, 2048], F16, name="pt", tag="pt")
            bdr = p2c_bd[u]
            src = bass.AP(bdr.tensor, bdr.offset + 128,
                          [[4 * BAND - 1, 128], [BAND, 4], [1, 512]])
            nc.gpsimd.dma_start(out=pt.rearrange("p (g j) -> p g j", g=4),
                                in_=src)

            if h == 0:
                ostage[b] = sb_out.tile([128, 512], F32, name=f"ostage{b}",
                                        tag="ostage")
            ps_pv = ps_pv_pool.tile([65, 512], F32, name="ps_pv", tag="ps_pv")
            eTs = []
            for half in range(2):
                ps_qk = ps_qk_pool.tile([128, 1024], F32,
                                        name="ps_qk", tag="ps_qk")
                for kl in range(2):
                    kb = half * 2 + kl
                    ks = slice(b * 512 + kb * 128, b * 512 + (kb + 1) * 128)
                    sl = slice(kl * 512, (kl + 1) * 512)
                    # PSUM = qk + c2p^T (4 transposed chunks) + p2c copy
                    nc.tensor.matmul(
                        out=ps_qk[:, sl], lhsT=k2T[hs, ks],
                        rhs=q2T[hs, b * 512:(b + 1) * 512],
                        start=True, stop=False)
                    for g in range(4):
                        nc.tensor.matmul(
                            out=ps_qk[:, kl * 512 + g * 128:
                                      kl * 512 + (g + 1) * 128],
                            lhsT=ct[:, g * 512 + kb * 128:
                                    g * 512 + kb * 128 + 128],
                            rhs=ident,
                            start=False, stop=False)
                    nc.tensor.matmul(
                        out=ps_qk[:, sl], lhsT=ident,
                        rhs=pt[:, kb * 512:(kb + 1) * 512],
                        start=False, stop=True)
                # one unnormalized exp for both kb halves, off PSUM
                eT = sb_work.tile([128, 1024], F16, name="eT", tag="eT",
                                  bufs=4)
                nc.scalar.activation(out=eT, in_=ps_qk, func=AF.Exp,
                                     scale=SCALE, bias=exp_bias)
                eTs.append(eT)
            for kb in range(4):
                nc.tensor.matmul(
                    out=ps_pv, lhsT=vtok[b * 4 + kb][:, h * 65:h * 65 + 65],
                    rhs=eTs[kb // 2][:, (kb % 2) * 512:(kb % 2 + 1) * 512],
                    start=(kb == 0), stop=(kb == 3))

            # --- finalize: out^T [65, 512] -> transpose -> /Z -> stage ---
            o2T = sb_work.tile([65, 512], F16, name="o2T", tag="o2T")
            nc.vector.tensor_copy(o2T, ps_pv)
            for qc in range(4):
                psT = ps_t_pool.tile([128, 65], F16, name="psT", tag="psT")
                nc.tensor.transpose(psT, o2T[:, qc * 128:(qc + 1) * 128],
                                    ident[0:65, 0:65])
                zrec = sb_work.tile([128, 1], F32, name="zrec",
                                    tag="zrec", bufs=8)
                nc.vector.reciprocal(zrec, psT[:, 64:65])
                nc.vector.tensor_scalar_mul(
                    ostage[b][:, qc * 128 + h * 64:qc * 128 + (h + 1) * 64],
                    psT[:, 0:64], zrec)
            if h == HC - 1:
                # one merged output write per batch
                dst = bass.AP(out_d.tensor, out_d.offset + b * 65536,
                              [[128, 128], [16384, 4], [1, 128]])
                nc.gpsimd.dma_start(
                    out=dst, in_=ostage[b].rearrange("p (g j) -> p g j", g=4))


_NC_CACHE = None


def _get_nc():
    global _NC_CACHE
    if _NC_CACHE is None:
        _NC_CACHE = build_nc()
    return _NC_CACHE


def make_in_maps(inputs):
    x = np.asarray(inputs["x"], np.float32)
    rel = np.asarray(inputs["rel_embeddings"], np.float32)
    Wq = np.asarray(inputs["Wq"], np.float32)
    Wk = np.asarray(inputs["Wk"], np.float32)
    Wv = np.asarray(inputs["Wv"], np.float32)
    bq = np.asarray(inputs["bq"], np.float32)
    bk = np.asarray(inputs["bk"], np.float32)
    bv = np.asarray(inputs["bv"], np.float32)

    xT = np.ascontiguousarray(x.reshape(T, DIM).T).astype(np.float16)
    relT = np.ascontiguousarray(rel[::-1].T).astype(np.float16)
    in_maps = []
    for c in range(NCORES):
        sl = slice(c * 128, (c + 1) * 128)
        in_maps.append({
            "xT": xT,
            "relT": relT,
            "Wq": np.ascontiguousarray(Wq[:, sl]).astype(np.float16),
            "Wk": np.ascontiguousarray(Wk[:, sl]).astype(np.float16),
            "Wv": np.ascontiguousarray(Wv[:, sl]).astype(np.float16),
            "bq": np.ascontiguousarray(bq[sl]).reshape(128, 1),
            "bk": np.ascontiguousarray(bk[sl]).reshape(128, 1),
            "bv": np.ascontiguousarray(bv[sl]).reshape(128, 1),
        })
    return in_maps


def kernel(**inputs):
    nc = _get_nc()
    in_maps = make_in_maps(inputs)
    res = run_bass_kernel_spmd(nc, in_maps, list(range(NCORES))).results
    out = np.concatenate([res[c]["out"] for c in range(NCORES)], axis=1)
    return out.reshape(B, S, DIM).astype(np.float32)


# revision 11
# speedup vs baseline: 1.6583x; 1.0587x over previous
"""Disentangled MHA (DeBERTa-style) Trainium2 Bass kernel, v2.

Sharding: 16 heads across 8 cores (2 heads/core), batch kept local.
Per core: project q/k/v with a 128-column weight slice, build banded
c2p/p2c score bands, round-trip them through DRAM as flat [128,2560]
tiles (one write + one 3D skew-read each), then fold the c2p transpose
and the p2c add directly into the QK PSUM with identity matmuls:
PSUM = qk + c2p^T + p2c, exp straight off PSUM, PV matmul with an
augmented [v|1] stationary for the fused softmax denominator.

B=4, S=512, DIM=1024, H=16, HD=64, MAX_REL=512.
"""

import numpy as np

import concourse.bass as bass
import concourse.bacc as bacc
import concourse.mybir as mybir
import concourse.tile as tile
from concourse.bass_utils import run_bass_kernel_spmd
from concourse.masks import make_identity

B, S, DIM, H, HD = 4, 512, 1024, 16, 64
T = B * S                      # 2048 tokens
R = 1024                       # 2 * att_span rel rows
HC = 2                         # heads per core
NCORES = 8
KC = DIM // 128                # contraction chunks
SCALE = float((HD * 3) ** (-0.5))
BAND = 640                     # skew band width (needs >= 512 + 127)

F32 = mybir.dt.float32
F16 = mybir.dt.float16
AF = mybir.ActivationFunctionType
ALU = mybir.AluOpType


def build_nc():
    nc = bacc.Bacc("TRN2", target_bir_lowering=False, debug=False)

    xT_d = nc.dram_tensor("xT", [DIM, T], F16, kind="ExternalInput")
    relT_d = nc.dram_tensor("relT", [DIM, R], F16, kind="ExternalInput")
    W_d = {
        n: nc.dram_tensor(f"W{n}", [DIM, 128], F16, kind="ExternalInput")
        for n in "qkv"
    }
    b_d = {
        n: nc.dram_tensor(f"b{n}", [128, 1], F32, kind="ExternalInput")
        for n in "qkv"
    }
    out_d = nc.dram_tensor("out", [T, 128], F32, kind="ExternalOutput")

    with tile.TileContext(nc) as tc:
        _body(nc, tc, xT_d.ap(), relT_d.ap(),
              {n: W_d[n].ap() for n in "qkv"},
              {n: b_d[n].ap() for n in "qkv"},
              out_d.ap())
    nc.compile()
    return nc


def _body(nc, tc, xT, relT, W, bvec, out_d):
    from contextlib import ExitStack
    ctx = ExitStack()
    with ctx:
        singles = ctx.enter_context(tc.tile_pool(name="singles", bufs=1))
        inputs_pool = tc.tile_pool(name="inputs", bufs=1)
        inp = inputs_pool.__enter__()

        # ---- Load inputs (spread across the three DMA queues) ----
        qeng = [nc.sync, nc.scalar, nc.gpsimd]
        xT_t = []
        for i in range(KC):
            t = inp.tile([128, T], F16, name=f"xT{i}")
            qeng[i % 3].dma_start(out=t, in_=xT[i * 128:(i + 1) * 128, :])
            xT_t.append(t)
        relT_t = []
        for i in range(KC):
            t = inp.tile([128, R], F16, name=f"relT{i}")
            qeng[(i + 1) % 3].dma_start(out=t, in_=relT[i * 128:(i + 1) * 128, :])
            relT_t.append(t)
        W_t = {}
        for wi, n in enumerate("qkv"):
            W_t[n] = []
            for i in range(KC):
                t = singles.tile([128, 128], F16, name=f"W{n}{i}")
                qeng[(i + wi) % 3].dma_start(
                    out=t, in_=W[n][i * 128:(i + 1) * 128, :])
                W_t[n].append(t)
        b_t = {}
        for n in "qkv":
            b_t[n] = singles.tile([128, 1], F32, name=f"b{n}")
            nc.gpsimd.dma_start(out=b_t[n], in_=bvec[n])

        ident = singles.tile([128, 128], F16, name="ident")
        make_identity(nc, ident)

        # ---- Phase A: projections (f16 matmuls, fp32 psum, f16 out) ----
        q2T = singles.tile([128, T], F16, name="q2T")
        k2T = singles.tile([128, T], F16, name="k2T")
        v2T = singles.tile([128, T], F16, name="v2T")
        posk = singles.tile([128, R], F16, name="posk")
        posq = singles.tile([128, R], F16, name="posq")

        projs = [
            (q2T, xT_t, "q", T),
            (k2T, xT_t, "k", T),
            (posk, relT_t, "k", R),
            (posq, relT_t, "q", R),
            (v2T, xT_t, "v", T),
        ]
        cast_flip = [0]
        with tc.tile_pool(name="psA", space="PSUM", bufs=2) as psA:
            for out_sb, rhs_tiles, wn, n_tot in projs:
                for nt in range(n_tot // 512):
                    ps = psA.tile([128, 512], F32, name="ps_proj", tag="ps_proj")
                    for kc in range(KC):
                        nc.tensor.matmul(
                            out=ps,
                            lhsT=W_t[wn][kc][:, :],
                            rhs=rhs_tiles[kc][:, nt * 512:(nt + 1) * 512],
                            start=(kc == 0), stop=(kc == KC - 1),
                        )
                    # cast f32->f16 + per-partition bias add (ACT/DVE split)
                    dst = out_sb[:, nt * 512:(nt + 1) * 512]
                    if cast_flip[0] % 2 == 0:
                        nc.scalar.activation(out=dst, in_=ps,
                                             func=AF.Identity, bias=b_t[wn],
                                             scale=1.0)
                    else:
                        nc.vector.tensor_scalar_add(dst, ps, b_t[wn])
                    cast_flip[0] += 1

            # ---- v_tok: transpose v2T to token-major, augmented ones col ----
            vtok = []
            for t in range(T // 128):
                vt = singles.tile([128, 130], F16, name=f"vtok{t}")
                vtok.append(vt)
            with tc.tile_pool(name="psVT", space="PSUM", bufs=2) as psVT:
                for t in range(T // 128):
                    pst = psVT.tile([128, 128], F16, name="ps_vt", tag="ps_vt")
                    nc.tensor.transpose(pst, v2T[:, t * 128:(t + 1) * 128], ident)
                    nc.vector.tensor_copy(vtok[t][:, 0:64], pst[:, 0:64])
                    nc.vector.tensor_copy(vtok[t][:, 65:129], pst[:, 64:128])
                    nc.gpsimd.memset(vtok[t][:, 64:65], 1.0)
                    nc.gpsimd.memset(vtok[t][:, 129:130], 1.0)

        # ---- zero-padded per-head q/k stationaries: K=64 matmuls stream
        # the moving operand at half rate, so pad the contraction to 128
        # (zeros in the other head's rows) and stream all 128 partitions.
        q2Tz = {}
        k2Tz = {}
        for h in range(HC):
            oh = slice((1 - h) * 64, (2 - h) * 64)
            hs = slice(h * 64, (h + 1) * 64)
            for nm, src, dstmap in (("q", q2T, q2Tz), ("k", k2T, k2Tz)):
                z = singles.tile([128, T], F16, name=f"{nm}2Tz{h}")
                nc.gpsimd.memset(z[oh, :], 0.0)
                nc.vector.tensor_copy(z[hs, :], src[hs, :])
                dstmap[h] = z

        inputs_pool.__exit__(None, None, None)

        # ---- Phase B pools ----
        band_dram = ctx.enter_context(
            tc.tile_pool(name="bands", space="DRAM", bufs=1))
        sb_band = ctx.enter_context(tc.tile_pool(name="sb_band", bufs=3))
        sb_read = ctx.enter_context(tc.tile_pool(name="sb_read", bufs=3))
        sb_work = ctx.enter_context(tc.tile_pool(name="sb_work", bufs=4))
        sb_out = ctx.enter_context(tc.tile_pool(name="sb_out", bufs=3))

        copy_flip = [0]

        def psum_to_sbuf_f16(dst, src):
            # alternate engines to balance DVE/ACT load
            if copy_flip[0] % 2 == 0:
                nc.vector.tensor_copy(dst, src)
            else:
                nc.scalar.copy(dst, src)
            copy_flip[0] += 1

        # constant exp bias keeps the f16 unnormalized-exp in range; it
        # cancels exactly in the final E@v / Z normalization
        exp_bias = singles.tile([128, 1], F32, name="exp_bias")
        nc.gpsimd.memset(exp_bias, -4.0)

        units = [(b, h) for b in range(B) for h in range(HC)]
        c2p_bd = {}
        p2c_bd = {}

        # --- B1: all band matmuls dense (keeps PE warm), writes stream out
        with tc.tile_pool(name="psBand", space="PSUM", bufs=2) as ps_band_pool:
            for u in range(len(units)):
                b, h = units[u]
                csb = sb_band.tile([128, 4 * BAND], F16, name=f"csb{u}",
                                   tag="csb")
                psb = sb_band.tile([128, 4 * BAND], F16, name=f"psb{u}",
                                   tag="psb")
                for g in range(4):
                    c0 = 128 * (3 - g)
                    cs = slice(b * 512 + g * 128, b * 512 + (g + 1) * 128)
                    for src2T, pos, stage in ((q2Tz[h], posk, csb),
                                              (k2Tz[h], posq, psb)):
                        ps = ps_band_pool.tile([128, BAND], F32,
                                               name="ps_band", tag="ps_band")
                        nc.tensor.matmul(
                            out=ps[:, 0:512], lhsT=src2T[:, cs],
                            rhs=pos[:, c0:c0 + 512], start=True, stop=True)
                        nc.tensor.matmul(
                            out=ps[:, 512:BAND], lhsT=src2T[:, cs],
                            rhs=pos[:, c0 + 512:c0 + BAND],
                            start=True, stop=True)
                        psum_to_sbuf_f16(stage[:, g * BAND:(g + 1) * BAND], ps)
                bdr = band_dram.tile([128, 4 * BAND], F16, name=f"c2pb_{u}",
                                     tag=f"c2p_dram_{u}", bufs=1)
                nc.sync.dma_start(out=bdr, in_=csb)
                c2p_bd[u] = bdr
                bdr = band_dram.tile([128, 4 * BAND], F16, name=f"p2cb_{u}",
                                     tag=f"p2c_dram_{u}", bufs=1)
                nc.gpsimd.dma_start(out=bdr, in_=psb)
                p2c_bd[u] = bdr

        # --- B2: skew reads + attention, [128,1024] two-bank psum halves
        ps_qk_pool = ctx.enter_context(
            tc.tile_pool(name="psQK", space="PSUM", bufs=2))
        ps_pv_pool = ctx.enter_context(
            tc.tile_pool(name="psPV", space="PSUM", bufs=2))
        ps_t_pool = ctx.enter_context(
            tc.tile_pool(name="psT", space="PSUM", bufs=2))
        ostage = {}

        for u in range(len(units)):
            b, h = units[u]
            # one 3D skew read per band: [p, g, j=128..639] with j=base+idx
            ct = sb_read.tile([128, 2048], F16, name="ct", tag="ct")
            bdr = c2p_bd[u]
            src = bass.AP(bdr.tensor, bdr.offset + 128,
                          [[4 * BAND - 1, 128], [BAND, 4], [1, 512]])
            nc.sync.dma_start(out=ct.rearrange("p (g j) -> p g j", g=4),
                              in_=src)
            pt = sb_read.tile([128, 2048], F16, name="pt", tag="pt")
            bdr = p2c_bd[u]
            src = bass.AP(bdr.tensor, bdr.offset + 128,
                          [[4 * BAND - 1, 128], [BAND, 4], [1, 512]])
            nc.gpsimd.dma_start(out=pt.rearrange("p (g j) -> p g j", g=4),
                                in_=src)

            if h == 0:
                ostage[b] = sb_out.tile([128, 512], F32, name=f"ostage{b}",
                                        tag="ostage")
            ps_pv = ps_pv_pool.tile([65, 512], F32, name="ps_pv", tag="ps_pv")
            eTs = []
            for half in range(2):
                ps_qk = ps_qk_pool.tile([128, 1024], F32,
                                        name="ps_qk", tag="ps_qk")
                for kl in range(2):
                    kb = half * 2 + kl
                    ks = slice(b * 512 + kb * 128, b * 512 + (kb + 1) * 128)
                    sl = slice(kl * 512, (kl + 1) * 512)
                    # PSUM = qk + c2p^T (4 transposed chunks) + p2c copy
                    nc.tensor.matmul(
                        out=ps_qk[:, sl], lhsT=k2Tz[h][:, ks],
                        rhs=q2T[:, b * 512:(b + 1) * 512],
                        start=True, stop=False)
                    for g in range(4):
                        nc.tensor.matmul(
                            out=ps_qk[:, kl * 512 + g * 128:
                                      kl * 512 + (g + 1) * 128],
                            lhsT=ct[:, g * 512 + kb * 128:
                                    g * 512 + kb * 128 + 128],
                            rhs=ident,
                            start=False, stop=False)
                    nc.tensor.matmul(
                        out=ps_qk[:, sl], lhsT=ident,
                        rhs=pt[:, kb * 512:(kb + 1) * 512],
                        start=False, stop=True)
                # one unnormalized exp for both kb halves, off PSUM
                eT = sb_work.tile([128, 1024], F16, name="eT", tag="eT",
                                  bufs=4)
                nc.scalar.activation(out=eT, in_=ps_qk, func=AF.Exp,
                                     scale=SCALE, bias=exp_bias)
                eTs.append(eT)
            for kb in range(4):
                nc.tensor.matmul(
                    out=ps_pv, lhsT=vtok[b * 4 + kb][:, h * 65:h * 65 + 65],
                    rhs=eTs[kb // 2][:, (kb % 2) * 512:(kb % 2 + 1) * 512],
                    start=(kb == 0), stop=(kb == 3))

            # --- finalize: out^T [65, 512] -> transpose -> /Z -> stage ---
            o2T = sb_work.tile([65, 512], F16, name="o2T", tag="o2T")
            nc.vector.tensor_copy(o2T, ps_pv)
            for qc in range(4):
                psT = ps_t_pool.tile([128, 65], F16, name="psT", tag="psT")
                nc.tensor.transpose(psT, o2T[:, qc * 128:(qc + 1) * 128],
                                    ident[0:65, 0:65])
                zrec = sb_work.tile([128, 1], F32, name="zrec",
                                    tag="zrec", bufs=8)
                nc.vector.reciprocal(zrec, psT[:, 64:65])
                nc.vector.tensor_scalar_mul(
                    ostage[b][:, qc * 128 + h * 64:qc * 128 + (h + 1) * 64],
                    psT[:, 0:64], zrec)
            if h == HC - 1:
                # one merged output write per batch
                dst = bass.AP(out_d.tensor, out_d.offset + b * 65536,
                              [[128, 128], [16384, 4], [1, 128]])
                nc.gpsimd.dma_start(
                    out=dst, in_=ostage[b].rearrange("p (g j) -> p g j", g=4))


_NC_CACHE = None


def _get_nc():
    global _NC_CACHE
    if _NC_CACHE is None:
        _NC_CACHE = build_nc()
    return _NC_CACHE


def make_in_maps(inputs):
    x = np.asarray(inputs["x"], np.float32)
    rel = np.asarray(inputs["rel_embeddings"], np.float32)
    Wq = np.asarray(inputs["Wq"], np.float32)
    Wk = np.asarray(inputs["Wk"], np.float32)
    Wv = np.asarray(inputs["Wv"], np.float32)
    bq = np.asarray(inputs["bq"], np.float32)
    bk = np.asarray(inputs["bk"], np.float32)
    bv = np.asarray(inputs["bv"], np.float32)

    xT = np.ascontiguousarray(x.reshape(T, DIM).T).astype(np.float16)
    relT = np.ascontiguousarray(rel[::-1].T).astype(np.float16)
    in_maps = []
    for c in range(NCORES):
        sl = slice(c * 128, (c + 1) * 128)
        in_maps.append({
            "xT": xT,
            "relT": relT,
            "Wq": np.ascontiguousarray(Wq[:, sl]).astype(np.float16),
            "Wk": np.ascontiguousarray(Wk[:, sl]).astype(np.float16),
            "Wv": np.ascontiguousarray(Wv[:, sl]).astype(np.float16),
            "bq": np.ascontiguousarray(bq[sl]).reshape(128, 1),
            "bk": np.ascontiguousarray(bk[sl]).reshape(128, 1),
            "bv": np.ascontiguousarray(bv[sl]).reshape(128, 1),
        })
    return in_maps


def kernel(**inputs):
    nc = _get_nc()
    in_maps = make_in_maps(inputs)
    res = run_bass_kernel_spmd(nc, in_maps, list(range(NCORES))).results
    out = np.concatenate([res[c]["out"] for c in range(NCORES)], axis=1)
    return out.reshape(B, S, DIM).astype(np.float32)


# revision 15
# speedup vs baseline: 1.7856x; 1.0768x over previous
"""Disentangled MHA (DeBERTa-style) Trainium2 Bass kernel, v2.

Sharding: 16 heads across 8 cores (2 heads/core), batch kept local.
Per core: project q/k/v with a 128-column weight slice, build banded
c2p/p2c score bands, round-trip them through DRAM as flat [128,2560]
tiles (one write + one 3D skew-read each), then fold the c2p transpose
and the p2c add directly into the QK PSUM with identity matmuls:
PSUM = qk + c2p^T + p2c, exp straight off PSUM, PV matmul with an
augmented [v|1] stationary for the fused softmax denominator.

B=4, S=512, DIM=1024, H=16, HD=64, MAX_REL=512.
"""

import numpy as np

import concourse.bass as bass
import concourse.bacc as bacc
import concourse.mybir as mybir
import concourse.tile as tile
from concourse.bass_utils import run_bass_kernel_spmd
from concourse.masks import make_identity

B, S, DIM, H, HD = 4, 512, 1024, 16, 64
T = B * S                      # 2048 tokens
R = 1024                       # 2 * att_span rel rows
HC = 2                         # heads per core
NCORES = 8
KC = DIM // 128                # contraction chunks
SCALE = float((HD * 3) ** (-0.5))
BAND = 640                     # skew band width (needs >= 512 + 127)

F32 = mybir.dt.float32
F16 = mybir.dt.float16
AF = mybir.ActivationFunctionType
ALU = mybir.AluOpType


def build_nc():
    nc = bacc.Bacc("TRN2", target_bir_lowering=False, debug=False)

    xT_d = nc.dram_tensor("xT", [DIM, T], F16, kind="ExternalInput")
    relT_d = nc.dram_tensor("relT", [DIM, R], F16, kind="ExternalInput")
    W_d = {
        n: nc.dram_tensor(f"W{n}", [DIM, 128], F16, kind="ExternalInput")
        for n in "qkv"
    }
    b_d = {
        n: nc.dram_tensor(f"b{n}", [128, 1], F32, kind="ExternalInput")
        for n in "qkv"
    }
    out_d = nc.dram_tensor("out", [T, 128], F32, kind="ExternalOutput")

    with tile.TileContext(nc) as tc:
        _body(nc, tc, xT_d.ap(), relT_d.ap(),
              {n: W_d[n].ap() for n in "qkv"},
              {n: b_d[n].ap() for n in "qkv"},
              out_d.ap())
    nc.compile()
    return nc


def _body(nc, tc, xT, relT, W, bvec, out_d):
    from contextlib import ExitStack
    ctx = ExitStack()
    with ctx:
        singles = ctx.enter_context(tc.tile_pool(name="singles", bufs=1))
        inputs_pool = tc.tile_pool(name="inputs", bufs=1)
        inp = inputs_pool.__enter__()

        # ---- Load inputs (spread across the three DMA queues) ----
        qeng = [nc.sync, nc.scalar, nc.gpsimd]
        xT_t = []
        for i in range(KC):
            t = inp.tile([128, T], F16, name=f"xT{i}")
            qeng[i % 3].dma_start(out=t, in_=xT[i * 128:(i + 1) * 128, :])
            xT_t.append(t)
        relT_t = []
        for i in range(KC):
            t = inp.tile([128, R], F16, name=f"relT{i}")
            qeng[(i + 1) % 3].dma_start(out=t, in_=relT[i * 128:(i + 1) * 128, :])
            relT_t.append(t)
        W_t = {}
        for wi, n in enumerate("qkv"):
            W_t[n] = []
            for i in range(KC):
                t = singles.tile([128, 128], F16, name=f"W{n}{i}")
                qeng[(i + wi) % 3].dma_start(
                    out=t, in_=W[n][i * 128:(i + 1) * 128, :])
                W_t[n].append(t)
        b_t = {}
        for n in "qkv":
            b_t[n] = singles.tile([128, 1], F32, name=f"b{n}")
            nc.gpsimd.dma_start(out=b_t[n], in_=bvec[n])

        ident = singles.tile([128, 128], F16, name="ident")
        make_identity(nc, ident)

        # ---- Phase A: projections (f16 matmuls, fp32 psum, f16 out) ----
        q2T = singles.tile([128, T], F16, name="q2T")
        k2T = singles.tile([128, T], F16, name="k2T")
        v2T = singles.tile([128, T], F16, name="v2T")
        posk = singles.tile([128, R], F16, name="posk")
        posq = singles.tile([128, R], F16, name="posq")

        projs = [
            (q2T, xT_t, "q", T),
            (k2T, xT_t, "k", T),
            (posk, relT_t, "k", R),
            (posq, relT_t, "q", R),
            (v2T, xT_t, "v", T),
        ]
        cast_flip = [0]
        with tc.tile_pool(name="psA", space="PSUM", bufs=2) as psA:
            for out_sb, rhs_tiles, wn, n_tot in projs:
                for nt in range(n_tot // 512):
                    ps = psA.tile([128, 512], F32, name="ps_proj", tag="ps_proj")
                    for kc in range(KC):
                        nc.tensor.matmul(
                            out=ps,
                            lhsT=W_t[wn][kc][:, :],
                            rhs=rhs_tiles[kc][:, nt * 512:(nt + 1) * 512],
                            start=(kc == 0), stop=(kc == KC - 1),
                        )
                    # cast f32->f16 + per-partition bias add (ACT/DVE split)
                    dst = out_sb[:, nt * 512:(nt + 1) * 512]
                    if cast_flip[0] % 2 == 0:
                        nc.scalar.activation(out=dst, in_=ps,
                                             func=AF.Identity, bias=b_t[wn],
                                             scale=1.0)
                    else:
                        nc.vector.tensor_scalar_add(dst, ps, b_t[wn])
                    cast_flip[0] += 1

            # ---- v_tok: transpose v2T to token-major, augmented ones col ----
            vtok = []
            for t in range(T // 128):
                vt = singles.tile([128, 130], F16, name=f"vtok{t}")
                vtok.append(vt)
            with tc.tile_pool(name="psVT", space="PSUM", bufs=2) as psVT:
                for t in range(T // 128):
                    pst = psVT.tile([128, 128], F16, name="ps_vt", tag="ps_vt")
                    nc.tensor.transpose(pst, v2T[:, t * 128:(t + 1) * 128], ident)
                    nc.vector.tensor_copy(vtok[t][:, 0:64], pst[:, 0:64])
                    nc.vector.tensor_copy(vtok[t][:, 65:129], pst[:, 64:128])
                    nc.gpsimd.memset(vtok[t][:, 64:65], 1.0)
                    nc.gpsimd.memset(vtok[t][:, 129:130], 1.0)

        # ---- zero-padded per-head q/k stationaries: K=64 matmuls stream
        # the moving operand at half rate, so pad the contraction to 128
        # (zeros in the other head's rows) and stream all 128 partitions.
        q2Tz = {}
        k2Tz = {}
        for h in range(HC):
            oh = slice((1 - h) * 64, (2 - h) * 64)
            hs = slice(h * 64, (h + 1) * 64)
            for nm, src, dstmap in (("q", q2T, q2Tz), ("k", k2T, k2Tz)):
                z = singles.tile([128, T], F16, name=f"{nm}2Tz{h}")
                nc.gpsimd.memset(z[oh, :], 0.0)
                nc.vector.tensor_copy(z[hs, :], src[hs, :])
                dstmap[h] = z

        inputs_pool.__exit__(None, None, None)

        # ---- Phase B pools ----
        band_dram = ctx.enter_context(
            tc.tile_pool(name="bands", space="DRAM", bufs=1))
        sb_band = ctx.enter_context(tc.tile_pool(name="sb_band", bufs=3))
        sb_read = ctx.enter_context(tc.tile_pool(name="sb_read", bufs=3))
        sb_work = ctx.enter_context(tc.tile_pool(name="sb_work", bufs=4))
        sb_out = ctx.enter_context(tc.tile_pool(name="sb_out", bufs=3))

        def psum_to_sbuf_f16(dst, src):
            # split across DVE+ACT so the psum slot frees ~2x sooner
            nc.vector.tensor_copy(dst[:, 0:320], src[:, 0:320])
            nc.scalar.copy(dst[:, 320:640], src[:, 320:640])

        # constant exp bias keeps the f16 unnormalized-exp in range; it
        # cancels exactly in the final E@v / Z normalization
        exp_bias = singles.tile([128, 1], F32, name="exp_bias")
        nc.gpsimd.memset(exp_bias, -4.0)

        units = [(b, h) for b in range(B) for h in range(HC)]
        c2p_bd = {}
        p2c_bd = {}

        # --- B1: all band matmuls dense (keeps PE warm), writes stream out
        with tc.tile_pool(name="psBand", space="PSUM", bufs=3) as ps_band_pool:
            for u in range(len(units)):
                b, h = units[u]
                csb = sb_band.tile([128, 4 * BAND], F16, name=f"csb{u}",
                                   tag="csb")
                psb = sb_band.tile([128, 4 * BAND], F16, name=f"psb{u}",
                                   tag="psb")
                for g in range(4):
                    c0 = 128 * (3 - g)
                    cs = slice(b * 512 + g * 128, b * 512 + (g + 1) * 128)
                    for src2T, pos, stage in ((q2Tz[h], posk, csb),
                                              (k2Tz[h], posq, psb)):
                        ps = ps_band_pool.tile([128, BAND], F32,
                                               name="ps_band", tag="ps_band")
                        nc.tensor.matmul(
                            out=ps[:, 0:512], lhsT=src2T[:, cs],
                            rhs=pos[:, c0:c0 + 512], start=True, stop=True)
                        nc.tensor.matmul(
                            out=ps[:, 512:BAND], lhsT=src2T[:, cs],
                            rhs=pos[:, c0 + 512:c0 + BAND],
                            start=True, stop=True)
                        psum_to_sbuf_f16(stage[:, g * BAND:(g + 1) * BAND], ps)
                bdr = band_dram.tile([128, 4 * BAND], F16, name=f"c2pb_{u}",
                                     tag=f"c2p_dram_{u}", bufs=1)
                nc.sync.dma_start(out=bdr, in_=csb)
                c2p_bd[u] = bdr
                bdr = band_dram.tile([128, 4 * BAND], F16, name=f"p2cb_{u}",
                                     tag=f"p2c_dram_{u}", bufs=1)
                nc.scalar.dma_start(out=bdr, in_=psb)
                p2c_bd[u] = bdr

        # --- B2: skew reads + attention, [128,1024] two-bank psum halves
        ps_qk_pool = ctx.enter_context(
            tc.tile_pool(name="psQK", space="PSUM", bufs=2))
        ps_pv_pool = ctx.enter_context(
            tc.tile_pool(name="psPV", space="PSUM", bufs=2))
        ps_t_pool = ctx.enter_context(
            tc.tile_pool(name="psT", space="PSUM", bufs=2))
        ostage = {}

        for u in range(len(units)):
            b, h = units[u]
            # one 3D skew read per band: [p, g, j=128..639] with j=base+idx
            ct = sb_read.tile([128, 2048], F16, name="ct", tag="ct")
            bdr = c2p_bd[u]
            src = bass.AP(bdr.tensor, bdr.offset + 128,
                          [[4 * BAND - 1, 128], [BAND, 4], [1, 512]])
            nc.sync.dma_start(out=ct.rearrange("p (g j) -> p g j", g=4),
                              in_=src)
            pt = sb_read.tile([128, 2048], F16, name="pt", tag="pt")
            bdr = p2c_bd[u]
            src = bass.AP(bdr.tensor, bdr.offset + 128,
                          [[4 * BAND - 1, 128], [BAND, 4], [1, 512]])
            nc.gpsimd.dma_start(out=pt.rearrange("p (g j) -> p g j", g=4),
                                in_=src)

            if h == 0:
                ostage[b] = sb_out.tile([128, 512], F32, name=f"ostage{b}",
                                        tag="ostage")
            ps_pv = ps_pv_pool.tile([65, 512], F32, name="ps_pv", tag="ps_pv")
            eTs = []
            for kb in range(4):
                ks = slice(b * 512 + kb * 128, b * 512 + (kb + 1) * 128)
                # PSUM = qk + c2p^T (4 transposed chunks) + p2c copy
                ps_qk = ps_qk_pool.tile([128, 512], F32,
                                        name="ps_qk", tag="ps_qk")
                nc.tensor.matmul(
                    out=ps_qk, lhsT=k2Tz[h][:, ks],
                    rhs=q2T[:, b * 512:(b + 1) * 512],
                    start=True, stop=False)
                for g in range(4):
                    nc.tensor.matmul(
                        out=ps_qk[:, g * 128:(g + 1) * 128],
                        lhsT=ct[:, g * 512 + kb * 128:
                                g * 512 + kb * 128 + 128],
                        rhs=ident,
                        start=False, stop=False)
                nc.tensor.matmul(
                    out=ps_qk, lhsT=ident,
                    rhs=pt[:, kb * 512:(kb + 1) * 512],
                    start=False, stop=True)
                # unnormalized exp straight off PSUM
                eT = sb_work.tile([128, 512], F16, name="eT", tag="eT",
                                  bufs=6)
                nc.scalar.activation(out=eT, in_=ps_qk, func=AF.Exp,
                                     scale=SCALE, bias=exp_bias)
                eTs.append(eT)
            for kb in range(4):
                nc.tensor.matmul(
                    out=ps_pv, lhsT=vtok[b * 4 + kb][:, h * 65:h * 65 + 65],
                    rhs=eTs[kb], start=(kb == 0), stop=(kb == 3))

            # --- finalize: out^T [65, 512] -> transpose -> /Z -> stage ---
            o2T = sb_work.tile([65, 512], F16, name="o2T", tag="o2T")
            nc.vector.tensor_copy(o2T, ps_pv)
            for qc in range(4):
                psT = ps_t_pool.tile([128, 65], F16, name="psT", tag="psT")
                nc.tensor.transpose(psT, o2T[:, qc * 128:(qc + 1) * 128],
                                    ident[0:65, 0:65])
                zrec = sb_work.tile([128, 1], F32, name="zrec",
                                    tag="zrec", bufs=8)
                nc.vector.reciprocal(zrec, psT[:, 64:65])
                nc.vector.tensor_scalar_mul(
                    ostage[b][:, qc * 128 + h * 64:qc * 128 + (h + 1) * 64],
                    psT[:, 0:64], zrec)
            if h == HC - 1:
                # one merged output write per batch
                dst = bass.AP(out_d.tensor, out_d.offset + b * 65536,
                              [[128, 128], [16384, 4], [1, 128]])
                nc.gpsimd.dma_start(
                    out=dst, in_=ostage[b].rearrange("p (g j) -> p g j", g=4))


_NC_CACHE = None


def _get_nc():
    global _NC_CACHE
    if _NC_CACHE is None:
        _NC_CACHE = build_nc()
    return _NC_CACHE


def make_in_maps(inputs):
    x = np.asarray(inputs["x"], np.float32)
    rel = np.asarray(inputs["rel_embeddings"], np.float32)
    Wq = np.asarray(inputs["Wq"], np.float32)
    Wk = np.asarray(inputs["Wk"], np.float32)
    Wv = np.asarray(inputs["Wv"], np.float32)
    bq = np.asarray(inputs["bq"], np.float32)
    bk = np.asarray(inputs["bk"], np.float32)
    bv = np.asarray(inputs["bv"], np.float32)

    xT = np.ascontiguousarray(x.reshape(T, DIM).T).astype(np.float16)
    relT = np.ascontiguousarray(rel[::-1].T).astype(np.float16)
    in_maps = []
    for c in range(NCORES):
        sl = slice(c * 128, (c + 1) * 128)
        in_maps.append({
            "xT": xT,
            "relT": relT,
            "Wq": np.ascontiguousarray(Wq[:, sl]).astype(np.float16),
            "Wk": np.ascontiguousarray(Wk[:, sl]).astype(np.float16),
            "Wv": np.ascontiguousarray(Wv[:, sl]).astype(np.float16),
            "bq": np.ascontiguousarray(bq[sl]).reshape(128, 1),
            "bk": np.ascontiguousarray(bk[sl]).reshape(128, 1),
            "bv": np.ascontiguousarray(bv[sl]).reshape(128, 1),
        })
    return in_maps


def kernel(**inputs):
    nc = _get_nc()
    in_maps = make_in_maps(inputs)
    res = run_bass_kernel_spmd(nc, in_maps, list(range(NCORES))).results
    out = np.concatenate([res[c]["out"] for c in range(NCORES)], axis=1)
    return out.reshape(B, S, DIM).astype(np.float32)
